# revision 74
# baseline (speedup 1.0000x reference)
"""Trainium2 Bass kernel for nn_Estor_concat (scatter_memory).

Math (exact reformulation of the reference):
  The attention output for a span of tag t is the per-tag constant
  v_tag[t] = out_proj(V_proj(tag_emb[t])) (softmax over one logit == 1),
  so the FFN input reduces to counts[t, s] * v_tag[t] concatenated over
  tags, and the first FFN layer collapses to the [T, H] weight-only
  constant W_eff[t, j] = sum_h v_tag[t, h] * ff1_w[j, t*H + h], folded on
  the host (constant folding, like BN-folding).  Per batch b the device
  computes:
    counts[t, s] = #spans(tag t) covering s
                 = sum_n oht[n,t]*(s >= start_n) - oht[n,t]*(s >= end_n)
    h1 = relu(W_eff.T @ counts + ff1_b)          [H, S]
    h2 = ff2 @ h1 + ff2_b                        [H2, S]  (fp8 DoubleRow)
    raw = [lwg_we | lwg_h2].T @ [we; h2]         [NL+1, S] (+ sum row)
    out = (raw - c1*mu + c2*sd) / bcast(sd)      (LayerNorm folded into
                                                  the output projection)
  with lwg = (lin_w * ln_g).T, c1 = col-sums of lwg, c2 = lin_w@ln_b+lin_b.

Sharding: pure data-parallel over batch (8 cores, 1 batch each), no
collectives; all post-fold weights are small and replicated.
"""

import ml_dtypes
import numpy as np

import concourse.bacc as bacc
import concourse.bass as bass
import concourse.mybir as mybir
import concourse.tile as tile
from concourse.bass_utils import run_bass_kernel_spmd

T, B, S, H = 16, 8, 512, 768
H2 = 384
NEW_H = H + H2          # 1152
NL = 33                 # num labels
NCORES = 8
KC_H = H // 128         # 6 chunks of the hidden dim
KC_H2 = H2 // 128       # 3
P = 128
M_PR = NL + 1           # 34: label rows + ones (sum) row
FF2_SCALE = 64.0        # fp8 pre-scale keeping ff2 out of e4m3 subnormals
H1_DIV = 4.0            # h1r stored /4 so prh2_w*4 clears fp8 subnormals
H2_SCALE = FF2_SCALE / H1_DIV   # h2 psum arrives scaled by this
LWG_W = KC_H * M_PR             # lwg_we chunks
F8_W = H2 + M_PR                # ff2 | prh2 packed width

F32 = mybir.dt.float32
BF16 = mybir.dt.bfloat16
F16 = mybir.dt.float16
F8 = mybir.dt.float8e4
DR = mybir.MatmulPerfMode.DoubleRow
ALU = mybir.AluOpType
ACT = mybir.ActivationFunctionType


def build_kernel(nt: int):
    nc = bacc.Bacc(
        "TRN2",
        target_bir_lowering=False,
        debug=False,
        enable_asserts=True,
        num_devices=NCORES,
    )

    def inp(name, shape, dtype=F32):
        return nc.dram_tensor(name, list(shape), dtype, kind="ExternalInput").ap()

    # packed inputs (few DMAs; see host prep for layouts)
    pk32 = inp("pk32", (P, 2 * nt + KC_H + KC_H2))  # sps | spe | ff1b | ff2b
    oht = inp("oht", (P, nt * 2 * T), BF16)         # [+onehot | -onehot]
    lwg = inp("lwg", (P, LWG_W), BF16)              # lwg_we lhsT chunks
    crow = inp("crow", (1, 3 * M_PR), F16)          # c1n | c2 | cb
    weff = inp("weff", (T, KC_H, P), BF16)          # W_eff[t, kj*128+m] / 4
    ff2t = inp("ff2t", (P, KC_H, F8_W), F8)         # ff2.T*64 | prh2_w.T*4
    we_t = inp("we_t", (P, KC_H, S), BF16)          # word_embedding[b].T

    out = nc.dram_tensor("out", [NL, S], F32, kind="ExternalOutput").ap()

    with tile.TileContext(nc) as tc:
        with (
            tc.tile_pool(name="singles", bufs=1) as singles,
            tc.tile_pool(name="spans", bufs=5) as spans,
            tc.tile_pool(name="ps_acc", bufs=1, space="PSUM") as ps_acc,
            tc.tile_pool(name="ps_h1", bufs=2, space="PSUM") as ps_h1,
            tc.tile_pool(name="ps_h2", bufs=2, space="PSUM") as ps_h2,
        ):
            # ---- constants ----
            ones_col = singles.tile([P, 1], BF16)
            nc.vector.memset(ones_col, 1.0)
            ones_row = singles.tile([1, M_PR], F16)
            nc.vector.memset(ones_row, 1.0)
            ones512 = singles.tile([1, S], F16)
            nc.vector.memset(ones512, 1.0)
            eps_t = singles.tile([1, 1], F32)
            nc.vector.memset(eps_t, 0.0)
            scratch = singles.tile([1, 1], F32)
            warm_sb = singles.tile([P, S], BF16)
            nc.gpsimd.memset(warm_sb, 0.25)
            # iota generated on-device: cheaper than a DMA (no 900ns sem)
            iota = singles.tile([P, S], F16)
            nc.gpsimd.iota(iota, [[1, S]], base=0, channel_multiplier=0,
                           allow_small_or_imprecise_dtypes=True)

            # ---- DMAs: mask-path + we lead the HWDGE queue; the small
            # weight tensors ride the Pool SWDGE queue in parallel ----
            pk32_sb = singles.tile([P, 2 * nt + KC_H + KC_H2], F32)
            nc.sync.dma_start(out=pk32_sb, in_=pk32)
            oht_sb = singles.tile([P, nt * 2 * T], BF16)
            nc.sync.dma_start(out=oht_sb, in_=oht)
            we_sb = singles.tile([P, KC_H, S], BF16)
            nc.sync.dma_start(out=we_sb[:, 0:3, :], in_=we_t[:, 0:3, :])
            nc.sync.dma_start(out=we_sb[:, 3:6, :], in_=we_t[:, 3:6, :])
            weff_sb = singles.tile([T, KC_H, P], BF16)
            nc.gpsimd.dma_start(out=weff_sb, in_=weff)
            lwg_sb = singles.tile([P, LWG_W], BF16)
            nc.gpsimd.dma_start(out=lwg_sb, in_=lwg)
            crow_sb = singles.tile([1, 3 * M_PR], F16)
            nc.gpsimd.dma_start(out=crow_sb, in_=crow)
            ff2_sb = singles.tile([P, KC_H, F8_W], F8)
            nc.gpsimd.dma_start(out=ff2_sb, in_=ff2t)

            def sps_col(i):
                return pk32_sb[:, i:i + 1]

            def spe_col(i):
                return pk32_sb[:, nt + i:nt + i + 1]

            def ff1b_col(kj):
                return pk32_sb[:, 2 * nt + kj:2 * nt + kj + 1]

            def ff2b_col(mc):
                return pk32_sb[:, 2 * nt + KC_H + mc:2 * nt + KC_H + mc + 1]

            def oht_pos(i):
                return oht_sb[:, i * 2 * T:i * 2 * T + T]

            def oht_neg(i):
                return oht_sb[:, i * 2 * T + T:(i + 1) * 2 * T]

            def lwg_c(fc):
                return lwg_sb[:, fc * M_PR:(fc + 1) * M_PR]

            c1n_row = crow_sb[0:1, 0:M_PR]
            c2_row = crow_sb[0:1, M_PR:2 * M_PR]
            cb_row = crow_sb[0:1, 2 * M_PR:3 * M_PR]

            # act-table warm-ups (loads overlap the DMA phase)
            nc.scalar.activation(out=scratch, in_=eps_t, func=ACT.Square)
            nc.scalar.activation(out=scratch, in_=eps_t, func=ACT.Sqrt,
                                 bias=eps_t)

            # ---- PE p-state warm-up: reach 2.4 GHz before real work ----
            warm_ps = ps_acc.tile([1, S], F32, tag="sdb")
            for _ in range(3):
                nc.tensor.matmul(warm_ps, ones_col, warm_sb,
                                 start=True, stop=True)

            # ---- counts: (s>=start) - (s>=end) scatter on PE ----
            counts_ps = ps_acc.tile([T, S], F32, tag="counts")
            for i in range(nt):
                ge_s = spans.tile([P, S], BF16, tag="ge_s")
                nc.vector.tensor_scalar(
                    out=ge_s, in0=iota, scalar1=sps_col(i), scalar2=None,
                    op0=ALU.is_ge,
                )
                ge_e = spans.tile([P, S], BF16, tag="ge_e")
                nc.vector.tensor_scalar(
                    out=ge_e, in0=iota, scalar1=spe_col(i), scalar2=None,
                    op0=ALU.is_ge,
                )
                nc.tensor.matmul(counts_ps, oht_pos(i), ge_s,
                                 start=(i == 0), stop=False)
                nc.tensor.matmul(counts_ps, oht_neg(i), ge_e,
                                 start=False, stop=(i == nt - 1))
            counts_sb = singles.tile([T, S], BF16)
            nc.vector.tensor_copy(out=counts_sb, in_=counts_ps)

            # ---- raw accumulation: constant cb opens the group ----
            pr_ps = ps_acc.tile([M_PR, S], F32, tag="pr")
            nc.tensor.matmul(pr_ps, cb_row, ones512, start=True, stop=False)
            for fc in range(3):
                nc.tensor.matmul(pr_ps, lwg_c(fc), we_sb[:, fc, :],
                                 start=False, stop=False)

            # ---- h1 = relu(W_eff.T @ counts + ff1_b) -> fp8 [H, S] ----
            h1r_sb = singles.tile([P, KC_H, S], F8)
            relu_eng = ["vec", "act", "vec", "act", "vec", "act"]
            h1_ps_l = []
            for kj in range(KC_H):
                if kj == 2:
                    # extra buffers via the freed counts and warm banks
                    ps = ps_acc.tile([P, S], F32, tag="counts")
                elif kj == 3:
                    ps = ps_acc.tile([P, S], F32, tag="sdb")
                else:
                    ps = ps_h1.tile([P, S], F32, tag="h1")
                h1_ps_l.append(ps)
                nc.tensor.matmul(ps, weff_sb[:, kj, :], counts_sb,
                                 start=True, stop=True)
            for kj in range(KC_H):
                ps = h1_ps_l[kj]
                if relu_eng[kj] == "vec":
                    nc.vector.tensor_scalar(
                        out=h1r_sb[:, kj, :], in0=ps, scalar1=ff1b_col(kj),
                        scalar2=0.0, op0=ALU.add, op1=ALU.max)
                elif relu_eng[kj] == "act":
                    nc.scalar.activation(
                        out=h1r_sb[:, kj, :], in_=ps, func=ACT.Relu,
                        bias=ff1b_col(kj))
                else:
                    nc.gpsimd.tensor_scalar(
                        out=h1r_sb[:, kj, :], in0=ps, scalar1=ff1b_col(kj),
                        scalar2=0.0, op0=ALU.add, op1=ALU.max)

            for fc in range(3, KC_H):
                nc.tensor.matmul(pr_ps, lwg_c(fc), we_sb[:, fc, :],
                                 start=False, stop=False)

            # ---- we squares ----
            sqp1 = singles.tile([P, 2, S], BF16)
            sqp2 = singles.tile([P, 2, S], BF16)
            sqw4 = singles.tile([P, S], BF16)
            sqw5 = singles.tile([P, S], BF16)
            nc.gpsimd.tensor_tensor(
                out=sqp1[:, 0, :], in0=we_sb[:, 0, :],
                in1=we_sb[:, 0, :], op=ALU.mult)
            nc.gpsimd.tensor_tensor(
                out=sqp1[:, 1, :], in0=we_sb[:, 1, :],
                in1=we_sb[:, 1, :], op=ALU.mult)
            nc.gpsimd.tensor_tensor(
                out=sqp2[:, 0, :], in0=we_sb[:, 2, :],
                in1=we_sb[:, 2, :], op=ALU.mult)
            nc.gpsimd.tensor_tensor(
                out=sqp2[:, 1, :], in0=we_sb[:, 3, :],
                in1=we_sb[:, 3, :], op=ALU.mult)
            nc.scalar.activation(
                out=sqw4, in_=we_sb[:, 4, :], func=ACT.Square)
            nc.scalar.activation(
                out=sqw5, in_=we_sb[:, 5, :], func=ACT.Square)

            # ---- sum of squares: tree-adds on the idle Pool engine ----
            a1 = singles.tile([P, S], BF16)
            nc.gpsimd.tensor_tensor(out=a1, in0=sqp1[:, 0, :],
                                    in1=sqp1[:, 1, :], op=ALU.add)
            a2 = singles.tile([P, S], BF16)
            nc.gpsimd.tensor_tensor(out=a2, in0=sqp2[:, 0, :],
                                    in1=sqp2[:, 1, :], op=ALU.add)
            a12 = singles.tile([P, S], BF16)
            nc.gpsimd.tensor_tensor(out=a12, in0=a1, in1=a2, op=ALU.add)
            a45 = singles.tile([P, S], BF16)
            nc.gpsimd.tensor_tensor(out=a45, in0=sqw4, in1=sqw5, op=ALU.add)
            acc_we = singles.tile([P, S], BF16)
            nc.gpsimd.tensor_tensor(out=acc_we, in0=a12, in1=a45,
                                    op=ALU.add)

            # ---- h2 = ff2 @ relu_h1 (fp8 DoubleRow) ----
            h2sqp = singles.tile([P, 3, S], BF16)
            h2_ps_l = []
            for mc in range(KC_H2):
                # third buffer for mc2 via the freed counts bank
                if mc == 2:
                    ps = ps_acc.tile([P, S], F32, tag="counts")
                else:
                    ps = ps_h2.tile([P, S], F32, tag="h2")
                h2_ps_l.append(ps)
            for mc in range(KC_H2):
                for kj in range(KC_H):
                    nc.tensor.matmul(
                        h2_ps_l[mc],
                        ff2_sb[:, kj, mc * P:(mc + 1) * P],
                        h1r_sb[:, kj, :],
                        start=(kj == 0), stop=(kj == KC_H - 1),
                    )
            # pr_h2 via folded fp8 weights: prh2_w.T @ relu(h1)
            for kj in range(KC_H):
                nc.tensor.matmul(
                    pr_ps, ff2_sb[:, kj, H2:H2 + M_PR],
                    h1r_sb[:, kj, :],
                    start=False, stop=(kj == KC_H - 1))

            # biased squares straight from psum (h2 values only feed the
            # variance; the label projection reads h1r via folded weights)
            nc.scalar.activation(
                out=h2sqp[:, 0, :], in_=h2_ps_l[0], func=ACT.Square,
                bias=ff2b_col(0), scale=1.0 / H2_SCALE)
            nc.scalar.activation(
                out=h2sqp[:, 1, :], in_=h2_ps_l[1], func=ACT.Square,
                bias=ff2b_col(1), scale=1.0 / H2_SCALE)
            nc.scalar.activation(
                out=h2sqp[:, 2, :], in_=h2_ps_l[2], func=ACT.Square,
                bias=ff2b_col(2), scale=1.0 / H2_SCALE)

            # ---- sum of squares, h2 part + single PE reduction ----
            b1 = singles.tile([P, S], BF16)
            nc.gpsimd.tensor_tensor(out=b1, in0=h2sqp[:, 0, :],
                                    in1=h2sqp[:, 1, :], op=ALU.add)
            b2 = singles.tile([P, S], BF16)
            nc.gpsimd.tensor_tensor(out=b2, in0=b1, in1=h2sqp[:, 2, :],
                                    op=ALU.add)
            comb = singles.tile([P, S], BF16)
            nc.vector.tensor_add(out=comb, in0=b2, in1=acc_we)
            ss_ps = ps_acc.tile([1, S], F32, tag="ss")
            nc.tensor.matmul(ss_ps, ones_col, comb, start=True, stop=True)

            # ---- LayerNorm stats: DVE back-to-back, no engine hops ----
            sumrow = singles.tile([1, S], F16)
            nc.vector.tensor_copy(out=sumrow[:, 0:S // 2],
                                  in_=pr_ps[0:1, 0:S // 2])
            nc.scalar.activation(out=sumrow[:, S // 2:S],
                                 in_=pr_ps[0:1, S // 2:S], func=ACT.Identity)
            # -c1*mu rides the pr psum accumulation (group re-opened)
            nc.tensor.matmul(pr_ps, c1n_row, sumrow,
                             start=False, stop=False, skip_group_check=True)
            mu = singles.tile([1, S], F16)
            nc.vector.tensor_scalar_mul(out=mu, in0=sumrow,
                                        scalar1=1.0 / NEW_H)
            mu2 = singles.tile([1, S], F16)
            nc.vector.tensor_mul(out=mu2, in0=mu, in1=mu)
            var_sb = singles.tile([1, S], F32)
            nc.vector.scalar_tensor_tensor(
                out=var_sb, in0=ss_ps, scalar=1.0 / NEW_H, in1=mu2,
                op0=ALU.mult, op1=ALU.subtract)
            sd = singles.tile([1, S], F16)
            nc.scalar.activation(out=sd, in_=var_sb, func=ACT.Sqrt,
                                 bias=eps_t)
            rvar = singles.tile([1, S], F32)
            nc.vector.reciprocal(out=rvar, in_=var_sb)
            # +c2*sd closes the pr group
            nc.tensor.matmul(pr_ps, c2_row, sd,
                             start=False, stop=True, skip_group_check=True)
            rstd = singles.tile([1, S], F32)
            nc.scalar.activation(out=rstd, in_=rvar, func=ACT.Sqrt,
                                 bias=eps_t)
            sdbs = singles.tile([M_PR, S], F32)
            nc.gpsimd.partition_broadcast(out_ap=sdbs, in_ap=rstd,
                                          channels=M_PR)

            # ---- final: (raw - c1*mu + c2*sd) / sd ----
            # asymmetric pieces on two DMA queues: the later piece is
            # smaller, so its divide (and the final semaphore) lands early
            HS = 416
            f_sa = singles.tile([M_PR, HS], F32)
            f_sb2 = singles.tile([M_PR, S - HS], F32)
            nc.vector.tensor_tensor(
                out=f_sa, in0=pr_ps[:, 0:HS],
                in1=sdbs[:, 0:HS], op=ALU.mult)
            nc.sync.dma_start(out=out[:, 0:HS], in_=f_sa[1:1 + NL, :])
            nc.vector.tensor_tensor(
                out=f_sb2, in0=pr_ps[:, HS:S],
                in1=sdbs[:, HS:S], op=ALU.mult)
            nc.scalar.dma_start(out=out[:, HS:S], in_=f_sb2[1:1 + NL, :])

    nc.compile()
    return nc


def _chunked(a, kc):
    """[kc*128, N...] -> [128, kc, N...] (partition-major chunk layout)."""
    return np.ascontiguousarray(
        a.reshape(kc, P, *a.shape[1:]).transpose(1, 0, *range(2, a.ndim + 1))
    )


_CACHE = {}


def kernel(**inputs) -> np.ndarray:
    bfl = ml_dtypes.bfloat16
    f8 = ml_dtypes.float8_e4m3
    we = np.asarray(inputs["word_embedding"], np.float32)
    te = np.asarray(inputs["tag_embedding"], np.float32)
    ipw = np.asarray(inputs["in_proj_w"], np.float32)
    ipb = np.asarray(inputs["in_proj_b"], np.float32)
    opw = np.asarray(inputs["out_proj_w"], np.float32)
    ob_ = np.asarray(inputs["out_proj_b"], np.float32)
    f1w = np.asarray(inputs["ff1_w"], np.float32)
    f1b = np.asarray(inputs["ff1_b"], np.float32)
    f2w = np.asarray(inputs["ff2_w"], np.float32)
    f2b = np.asarray(inputs["ff2_b"], np.float32)
    lg = np.asarray(inputs["ln_g"], np.float32)
    lb = np.asarray(inputs["ln_b"], np.float32)
    lw = np.asarray(inputs["lin_w"], np.float32)
    lbias = np.asarray(inputs["lin_b"], np.float32)
    sb = np.asarray(inputs["span_batch"]).astype(np.int64)
    st = np.asarray(inputs["span_tag"]).astype(np.int64)
    ss = np.asarray(inputs["span_start"]).astype(np.int64)
    se = np.asarray(inputs["span_end"]).astype(np.int64)

    # ---- weight-only constant folding (host) --------------------------
    v_tag = (te @ ipw[2 * H:].T + ipb[2 * H:]) @ opw.T + ob_   # [T, H]
    weff = np.stack(
        [f1w[:, t * H:(t + 1) * H] @ v_tag[t] for t in range(T)])
    weff_c = np.ascontiguousarray(
        (weff / H1_DIV).reshape(T, KC_H, P).astype(bfl))

    lwgT = (lw * lg).T                                   # [NEW_H, NL]
    lwg_np = np.zeros((P, KC_H, M_PR), bfl)              # we-part lhsT
    lwg_np[:, :, 0] = 1.0                                # sum row first
    lwg_np[:, :, 1:] = _chunked(lwgT[:H].astype(bfl), KC_H)
    c1n_np = np.zeros(M_PR, np.float16)
    c1n_np[1:] = (-lwgT.sum(0) / NEW_H).astype(np.float16)
    c2_np = np.zeros(M_PR, np.float16)
    c2_np[1:] = (lw @ lb + lbias).astype(np.float16)
    # fold lwg_h2.T @ ff2: the label projection reads relu(h1) directly
    lwg_h2 = np.concatenate(
        [np.ones((H2, 1), np.float32), lwgT[H:]], axis=1)    # [H2, 34]
    prh2_full = lwg_h2.T @ f2w                               # [34, H]
    cb_np = (lwg_h2.T @ f2b).astype(np.float16)              # [34]

    ff2t_np = np.zeros((P, KC_H, F8_W), f8)
    ff2t_np[:, :, :H2] = _chunked((f2w.T * FF2_SCALE).astype(f8), KC_H)
    ff2t_np[:, :, H2:] = _chunked(
        np.ascontiguousarray(prh2_full.T * H1_DIV).astype(f8), KC_H)
    ff1b_np = np.ascontiguousarray(f1b.reshape(KC_H, P).T) / H1_DIV
    ff2b_np = np.ascontiguousarray(f2b.reshape(KC_H2, P).T)

    counts_per_b = np.bincount(sb, minlength=B)
    nt = max(1, int(np.ceil(counts_per_b.max() / P)))
    n_pad = nt * P

    in_maps = []
    for c in range(NCORES):
        idx = np.where(sb == c)[0]
        n = len(idx)
        pk32 = np.zeros((P, 2 * nt + KC_H + KC_H2), np.float32)
        sps_np = np.zeros(n_pad, np.float32)
        spe_np = np.zeros(n_pad, np.float32)
        oht_np = np.zeros((n_pad, 2 * T), bfl)
        sps_np[:n] = ss[idx]
        spe_np[:n] = se[idx]
        oht_np[np.arange(n), st[idx]] = 1.0
        oht_np[np.arange(n), T + st[idx]] = -1.0
        pk32[:, 0:nt] = sps_np.reshape(nt, P).T
        pk32[:, nt:2 * nt] = spe_np.reshape(nt, P).T
        pk32[:, 2 * nt:2 * nt + KC_H] = ff1b_np
        pk32[:, 2 * nt + KC_H:] = ff2b_np
        oht_pk = np.ascontiguousarray(
            oht_np.reshape(nt, P, 2 * T).transpose(1, 0, 2)
            .reshape(P, nt * 2 * T))
        lwg_pk = np.ascontiguousarray(lwg_np.reshape(P, LWG_W))
        crow_pk = np.zeros((1, 3 * M_PR), np.float16)
        crow_pk[0, 0:M_PR] = c1n_np
        crow_pk[0, M_PR:2 * M_PR] = c2_np
        crow_pk[0, 2 * M_PR:] = cb_np
        in_maps.append(dict(
            pk32=pk32, oht=oht_pk, lwg=lwg_pk, crow=crow_pk,
            weff=weff_c, ff2t=ff2t_np,
            we_t=_chunked(np.ascontiguousarray(we[c].T).astype(bfl), KC_H),
        ))

    if nt not in _CACHE:
        _CACHE[nt] = build_kernel(nt)
    nc = _CACHE[nt]

    res = run_bass_kernel_spmd(nc, in_maps, list(range(NCORES)))
    out = np.stack([res.results[c]["out"].T for c in range(NCORES)])
    return out.astype(np.float32)


if __name__ == "__main__":
    import reference
    inp = {k: np.asarray(v) for k, v in reference.setup_inputs().items()}
    got = kernel(**inp)
    print("kernel output:", got.shape, got.dtype)


# revision 75
# speedup vs baseline: 1.0162x; 1.0162x over previous
"""Trainium2 Bass kernel for nn_Estor_concat (scatter_memory).

Math (exact reformulation of the reference):
  The attention output for a span of tag t is the per-tag constant
  v_tag[t] = out_proj(V_proj(tag_emb[t])) (softmax over one logit == 1),
  so the FFN input reduces to counts[t, s] * v_tag[t] concatenated over
  tags, and the first FFN layer collapses to the [T, H] weight-only
  constant W_eff[t, j] = sum_h v_tag[t, h] * ff1_w[j, t*H + h], folded on
  the host (constant folding, like BN-folding).  Per batch b the device
  computes:
    counts[t, s] = #spans(tag t) covering s
                 = sum_n oht[n,t]*(s >= start_n) - oht[n,t]*(s >= end_n)
    h1 = relu(W_eff.T @ counts + ff1_b)          [H, S]
    h2 = ff2 @ h1 + ff2_b                        [H2, S]  (fp8 DoubleRow)
    raw = [lwg_we | lwg_h2].T @ [we; h2]         [NL+1, S] (+ sum row)
    out = (raw - c1*mu + c2*sd) / bcast(sd)      (LayerNorm folded into
                                                  the output projection)
  with lwg = (lin_w * ln_g).T, c1 = col-sums of lwg, c2 = lin_w@ln_b+lin_b.

Sharding: pure data-parallel over batch (8 cores, 1 batch each), no
collectives; all post-fold weights are small and replicated.
"""

import ml_dtypes
import numpy as np

import concourse.bacc as bacc
import concourse.bass as bass
import concourse.mybir as mybir
import concourse.tile as tile
from concourse.bass_utils import run_bass_kernel_spmd

T, B, S, H = 16, 8, 512, 768
H2 = 384
NEW_H = H + H2          # 1152
NL = 33                 # num labels
NCORES = 8
KC_H = H // 128         # 6 chunks of the hidden dim
KC_H2 = H2 // 128       # 3
P = 128
M_PR = NL + 1           # 34: label rows + ones (sum) row
FF2_SCALE = 64.0        # fp8 pre-scale keeping ff2 out of e4m3 subnormals
H1_DIV = 4.0            # h1r stored /4 so prh2_w*4 clears fp8 subnormals
H2_SCALE = FF2_SCALE / H1_DIV   # h2 psum arrives scaled by this
LWG_W = KC_H * M_PR             # lwg_we chunks
F8_W = H2 + M_PR                # ff2 | prh2 packed width

F32 = mybir.dt.float32
BF16 = mybir.dt.bfloat16
F16 = mybir.dt.float16
F8 = mybir.dt.float8e4
DR = mybir.MatmulPerfMode.DoubleRow
ALU = mybir.AluOpType
ACT = mybir.ActivationFunctionType


def build_kernel(nt: int):
    nc = bacc.Bacc(
        "TRN2",
        target_bir_lowering=False,
        debug=False,
        enable_asserts=True,
        num_devices=NCORES,
    )

    def inp(name, shape, dtype=F32):
        return nc.dram_tensor(name, list(shape), dtype, kind="ExternalInput").ap()

    # packed inputs (few DMAs; see host prep for layouts)
    pk32 = inp("pk32", (P, 2 * nt + KC_H + KC_H2))  # sps | spe | ff1b | ff2b
    oht = inp("oht", (P, nt * 2 * T), BF16)         # [+onehot | -onehot]
    lwg = inp("lwg", (P, LWG_W), BF16)              # lwg_we lhsT chunks
    crow = inp("crow", (1, 3 * M_PR), F16)          # c1n | c2 | cb
    weff = inp("weff", (T, KC_H, P), BF16)          # W_eff[t, kj*128+m] / 4
    ff2t = inp("ff2t", (P, KC_H, F8_W), F8)         # ff2.T*64 | prh2_w.T*4
    we_t = inp("we_t", (P, KC_H, S), BF16)          # word_embedding[b].T

    out = nc.dram_tensor("out", [NL, S], F32, kind="ExternalOutput").ap()

    with tile.TileContext(nc) as tc:
        with (
            tc.tile_pool(name="singles", bufs=1) as singles,
            tc.tile_pool(name="spans", bufs=5) as spans,
            tc.tile_pool(name="ps_acc", bufs=1, space="PSUM") as ps_acc,
            tc.tile_pool(name="ps_h1", bufs=2, space="PSUM") as ps_h1,
            tc.tile_pool(name="ps_h2", bufs=2, space="PSUM") as ps_h2,
        ):
            # ---- constants ----
            ones_col = singles.tile([P, 1], BF16)
            nc.vector.memset(ones_col, 1.0)
            ones_row = singles.tile([1, M_PR], F16)
            nc.vector.memset(ones_row, 1.0)
            ones512 = singles.tile([1, S], F16)
            nc.vector.memset(ones512, 1.0)
            eps_t = singles.tile([1, 1], F32)
            nc.vector.memset(eps_t, 0.0)
            scratch = singles.tile([1, 1], F32)
            warm_sb = singles.tile([P, S], BF16)
            nc.gpsimd.memset(warm_sb, 0.25)
            # iota generated on-device: cheaper than a DMA (no 900ns sem)
            iota = singles.tile([P, S], F16)
            nc.gpsimd.iota(iota, [[1, S]], base=0, channel_multiplier=0,
                           allow_small_or_imprecise_dtypes=True)

            # ---- DMAs: mask-path + we lead the HWDGE queue; the small
            # weight tensors ride the Pool SWDGE queue in parallel ----
            pk32_sb = singles.tile([P, 2 * nt + KC_H + KC_H2], F32)
            nc.sync.dma_start(out=pk32_sb, in_=pk32)
            oht_sb = singles.tile([P, nt * 2 * T], BF16)
            nc.sync.dma_start(out=oht_sb, in_=oht)
            we_sb = singles.tile([P, KC_H, S], BF16)
            nc.sync.dma_start(out=we_sb[:, 0:3, :], in_=we_t[:, 0:3, :])
            nc.sync.dma_start(out=we_sb[:, 3:6, :], in_=we_t[:, 3:6, :])
            weff_sb = singles.tile([T, KC_H, P], BF16)
            nc.gpsimd.dma_start(out=weff_sb, in_=weff)
            lwg_sb = singles.tile([P, LWG_W], BF16)
            nc.gpsimd.dma_start(out=lwg_sb, in_=lwg)
            crow_sb = singles.tile([1, 3 * M_PR], F16)
            nc.gpsimd.dma_start(out=crow_sb, in_=crow)
            ff2_sb = singles.tile([P, KC_H, F8_W], F8)
            nc.gpsimd.dma_start(out=ff2_sb, in_=ff2t)

            def sps_col(i):
                return pk32_sb[:, i:i + 1]

            def spe_col(i):
                return pk32_sb[:, nt + i:nt + i + 1]

            def ff1b_col(kj):
                return pk32_sb[:, 2 * nt + kj:2 * nt + kj + 1]

            def ff2b_col(mc):
                return pk32_sb[:, 2 * nt + KC_H + mc:2 * nt + KC_H + mc + 1]

            def oht_pos(i):
                return oht_sb[:, i * 2 * T:i * 2 * T + T]

            def oht_neg(i):
                return oht_sb[:, i * 2 * T + T:(i + 1) * 2 * T]

            def lwg_c(fc):
                return lwg_sb[:, fc * M_PR:(fc + 1) * M_PR]

            c1n_row = crow_sb[0:1, 0:M_PR]
            c2_row = crow_sb[0:1, M_PR:2 * M_PR]
            cb_row = crow_sb[0:1, 2 * M_PR:3 * M_PR]

            # act-table warm-ups (loads overlap the DMA phase)
            nc.scalar.activation(out=scratch, in_=eps_t, func=ACT.Square)
            nc.scalar.activation(out=scratch, in_=eps_t, func=ACT.Sqrt,
                                 bias=eps_t)

            # ---- PE p-state warm-up: reach 2.4 GHz before real work ----
            warm_ps = ps_acc.tile([1, S], F32, tag="sdb")
            for _ in range(3):
                nc.tensor.matmul(warm_ps, ones_col, warm_sb,
                                 start=True, stop=True)

            # ---- counts: (s>=start) - (s>=end) scatter on PE ----
            counts_ps = ps_acc.tile([T, S], F32, tag="counts")
            for i in range(nt):
                ge_s = spans.tile([P, S], BF16, tag="ge_s")
                nc.vector.tensor_scalar(
                    out=ge_s, in0=iota, scalar1=sps_col(i), scalar2=None,
                    op0=ALU.is_ge,
                )
                ge_e = spans.tile([P, S], BF16, tag="ge_e")
                nc.vector.tensor_scalar(
                    out=ge_e, in0=iota, scalar1=spe_col(i), scalar2=None,
                    op0=ALU.is_ge,
                )
                nc.tensor.matmul(counts_ps, oht_pos(i), ge_s,
                                 start=(i == 0), stop=False)
                nc.tensor.matmul(counts_ps, oht_neg(i), ge_e,
                                 start=False, stop=(i == nt - 1))
            counts_sb = singles.tile([T, S], BF16)
            nc.vector.tensor_copy(out=counts_sb, in_=counts_ps)

            # ---- raw accumulation: constant cb opens the group ----
            pr_ps = ps_acc.tile([M_PR, S], F32, tag="pr")
            nc.tensor.matmul(pr_ps, cb_row, ones512, start=True, stop=False)
            for fc in range(3):
                nc.tensor.matmul(pr_ps, lwg_c(fc), we_sb[:, fc, :],
                                 start=False, stop=False)

            # ---- h1 = relu(W_eff.T @ counts + ff1_b) -> fp8 [H, S] ----
            h1r_sb = singles.tile([P, KC_H, S], F8)
            relu_eng = ["vec", "act", "vec", "act", "vec", "act"]
            h1_ps_l = []
            for kj in range(KC_H):
                if kj == 2:
                    # extra buffers via the freed counts and warm banks
                    ps = ps_acc.tile([P, S], F32, tag="counts")
                elif kj == 3:
                    ps = ps_acc.tile([P, S], F32, tag="sdb")
                else:
                    ps = ps_h1.tile([P, S], F32, tag="h1")
                h1_ps_l.append(ps)
                nc.tensor.matmul(ps, weff_sb[:, kj, :], counts_sb,
                                 start=True, stop=True)
            for kj in range(KC_H):
                ps = h1_ps_l[kj]
                if relu_eng[kj] == "vec":
                    nc.vector.tensor_scalar(
                        out=h1r_sb[:, kj, :], in0=ps, scalar1=ff1b_col(kj),
                        scalar2=0.0, op0=ALU.add, op1=ALU.max)
                elif relu_eng[kj] == "act":
                    nc.scalar.activation(
                        out=h1r_sb[:, kj, :], in_=ps, func=ACT.Relu,
                        bias=ff1b_col(kj))
                else:
                    nc.gpsimd.tensor_scalar(
                        out=h1r_sb[:, kj, :], in0=ps, scalar1=ff1b_col(kj),
                        scalar2=0.0, op0=ALU.add, op1=ALU.max)

            for fc in range(3, KC_H):
                nc.tensor.matmul(pr_ps, lwg_c(fc), we_sb[:, fc, :],
                                 start=False, stop=False)

            # ---- we squares ----
            sqp1 = singles.tile([P, 2, S], BF16)
            sqp2 = singles.tile([P, 2, S], BF16)
            sqw4 = singles.tile([P, S], BF16)
            sqw5 = singles.tile([P, S], BF16)
            nc.gpsimd.tensor_tensor(
                out=sqp1[:, 0, :], in0=we_sb[:, 0, :],
                in1=we_sb[:, 0, :], op=ALU.mult)
            nc.gpsimd.tensor_tensor(
                out=sqp1[:, 1, :], in0=we_sb[:, 1, :],
                in1=we_sb[:, 1, :], op=ALU.mult)
            nc.gpsimd.tensor_tensor(
                out=sqp2[:, 0, :], in0=we_sb[:, 2, :],
                in1=we_sb[:, 2, :], op=ALU.mult)
            nc.gpsimd.tensor_tensor(
                out=sqp2[:, 1, :], in0=we_sb[:, 3, :],
                in1=we_sb[:, 3, :], op=ALU.mult)
            nc.scalar.activation(
                out=sqw4, in_=we_sb[:, 4, :], func=ACT.Square)
            nc.scalar.activation(
                out=sqw5, in_=we_sb[:, 5, :], func=ACT.Square)

            # ---- sum of squares: tree-adds on the idle Pool engine ----
            a1 = singles.tile([P, S], BF16)
            nc.gpsimd.tensor_tensor(out=a1, in0=sqp1[:, 0, :],
                                    in1=sqp1[:, 1, :], op=ALU.add)
            a2 = singles.tile([P, S], BF16)
            nc.gpsimd.tensor_tensor(out=a2, in0=sqp2[:, 0, :],
                                    in1=sqp2[:, 1, :], op=ALU.add)
            a12 = singles.tile([P, S], BF16)
            nc.gpsimd.tensor_tensor(out=a12, in0=a1, in1=a2, op=ALU.add)
            a45 = singles.tile([P, S], BF16)
            nc.gpsimd.tensor_tensor(out=a45, in0=sqw4, in1=sqw5, op=ALU.add)
            acc_we = singles.tile([P, S], BF16)
            nc.gpsimd.tensor_tensor(out=acc_we, in0=a12, in1=a45,
                                    op=ALU.add)

            # ---- h2 = ff2 @ relu_h1 (fp8 DoubleRow) ----
            h2sqp = singles.tile([P, 3, S], BF16)
            h2_ps_l = []
            for mc in range(KC_H2):
                # third buffer for mc2 via the freed counts bank
                if mc == 2:
                    ps = ps_acc.tile([P, S], F32, tag="counts")
                else:
                    ps = ps_h2.tile([P, S], F32, tag="h2")
                h2_ps_l.append(ps)
            for mc in range(KC_H2):
                for kj in range(KC_H):
                    nc.tensor.matmul(
                        h2_ps_l[mc],
                        ff2_sb[:, kj, mc * P:(mc + 1) * P],
                        h1r_sb[:, kj, :],
                        start=(kj == 0), stop=(kj == KC_H - 1),
                    )
            # pr_h2 via folded fp8 weights: prh2_w.T @ relu(h1)
            for kj in range(KC_H):
                nc.tensor.matmul(
                    pr_ps, ff2_sb[:, kj, H2:H2 + M_PR],
                    h1r_sb[:, kj, :],
                    start=False, stop=(kj == KC_H - 1))

            # biased squares straight from psum (h2 values only feed the
            # variance; the label projection reads h1r via folded weights)
            nc.scalar.activation(
                out=h2sqp[:, 0, :], in_=h2_ps_l[0], func=ACT.Square,
                bias=ff2b_col(0), scale=1.0 / H2_SCALE)
            nc.scalar.activation(
                out=h2sqp[:, 1, :], in_=h2_ps_l[1], func=ACT.Square,
                bias=ff2b_col(1), scale=1.0 / H2_SCALE)
            nc.scalar.activation(
                out=h2sqp[:, 2, :], in_=h2_ps_l[2], func=ACT.Square,
                bias=ff2b_col(2), scale=1.0 / H2_SCALE)

            # ---- sum of squares, h2 part + single PE reduction ----
            b1 = singles.tile([P, S], BF16)
            nc.gpsimd.tensor_tensor(out=b1, in0=h2sqp[:, 0, :],
                                    in1=h2sqp[:, 1, :], op=ALU.add)
            b2 = singles.tile([P, S], BF16)
            nc.gpsimd.tensor_tensor(out=b2, in0=b1, in1=h2sqp[:, 2, :],
                                    op=ALU.add)
            comb = singles.tile([P, S], BF16)
            nc.vector.tensor_add(out=comb, in0=b2, in1=acc_we)
            ss_ps = ps_acc.tile([1, S], F32, tag="ss")
            nc.tensor.matmul(ss_ps, ones_col, comb, start=True, stop=True)

            # ---- LayerNorm stats: DVE back-to-back, no engine hops ----
            sumrow = singles.tile([1, S], F16)
            nc.vector.tensor_copy(out=sumrow, in_=pr_ps[0:1, :])
            # -c1*mu rides the pr psum accumulation (group re-opened)
            nc.tensor.matmul(pr_ps, c1n_row, sumrow,
                             start=False, stop=False, skip_group_check=True)
            mu = singles.tile([1, S], F16)
            nc.vector.tensor_scalar_mul(out=mu, in0=sumrow,
                                        scalar1=1.0 / NEW_H)
            mu2 = singles.tile([1, S], F16)
            nc.vector.tensor_mul(out=mu2, in0=mu, in1=mu)
            var_sb = singles.tile([1, S], F32)
            nc.vector.scalar_tensor_tensor(
                out=var_sb, in0=ss_ps, scalar=1.0 / NEW_H, in1=mu2,
                op0=ALU.mult, op1=ALU.subtract)
            sd = singles.tile([1, S], F16)
            nc.scalar.activation(out=sd, in_=var_sb, func=ACT.Sqrt,
                                 bias=eps_t)
            rvar = singles.tile([1, S], F32)
            nc.vector.reciprocal(out=rvar, in_=var_sb)
            # +c2*sd closes the pr group
            nc.tensor.matmul(pr_ps, c2_row, sd,
                             start=False, stop=True, skip_group_check=True)
            rstd = singles.tile([1, S], F32)
            nc.scalar.activation(out=rstd, in_=rvar, func=ACT.Sqrt,
                                 bias=eps_t)
            sdbs = singles.tile([M_PR, S], F32)
            nc.gpsimd.partition_broadcast(out_ap=sdbs, in_ap=rstd,
                                          channels=M_PR)

            # ---- final: (raw - c1*mu + c2*sd) / sd ----
            # asymmetric pieces on two DMA queues: the later piece is
            # smaller, so its divide (and the final semaphore) lands early
            HS = 416
            f_sa = singles.tile([M_PR, HS], F32)
            f_sb2 = singles.tile([M_PR, S - HS], F32)
            nc.vector.tensor_tensor(
                out=f_sa, in0=pr_ps[:, 0:HS],
                in1=sdbs[:, 0:HS], op=ALU.mult)
            nc.sync.dma_start(out=out[:, 0:HS], in_=f_sa[1:1 + NL, :])
            nc.vector.tensor_tensor(
                out=f_sb2, in0=pr_ps[:, HS:S],
                in1=sdbs[:, HS:S], op=ALU.mult)
            nc.scalar.dma_start(out=out[:, HS:S], in_=f_sb2[1:1 + NL, :])

    nc.compile()
    return nc


def _chunked(a, kc):
    """[kc*128, N...] -> [128, kc, N...] (partition-major chunk layout)."""
    return np.ascontiguousarray(
        a.reshape(kc, P, *a.shape[1:]).transpose(1, 0, *range(2, a.ndim + 1))
    )


_CACHE = {}


def kernel(**inputs) -> np.ndarray:
    bfl = ml_dtypes.bfloat16
    f8 = ml_dtypes.float8_e4m3
    we = np.asarray(inputs["word_embedding"], np.float32)
    te = np.asarray(inputs["tag_embedding"], np.float32)
    ipw = np.asarray(inputs["in_proj_w"], np.float32)
    ipb = np.asarray(inputs["in_proj_b"], np.float32)
    opw = np.asarray(inputs["out_proj_w"], np.float32)
    ob_ = np.asarray(inputs["out_proj_b"], np.float32)
    f1w = np.asarray(inputs["ff1_w"], np.float32)
    f1b = np.asarray(inputs["ff1_b"], np.float32)
    f2w = np.asarray(inputs["ff2_w"], np.float32)
    f2b = np.asarray(inputs["ff2_b"], np.float32)
    lg = np.asarray(inputs["ln_g"], np.float32)
    lb = np.asarray(inputs["ln_b"], np.float32)
    lw = np.asarray(inputs["lin_w"], np.float32)
    lbias = np.asarray(inputs["lin_b"], np.float32)
    sb = np.asarray(inputs["span_batch"]).astype(np.int64)
    st = np.asarray(inputs["span_tag"]).astype(np.int64)
    ss = np.asarray(inputs["span_start"]).astype(np.int64)
    se = np.asarray(inputs["span_end"]).astype(np.int64)

    # ---- weight-only constant folding (host) --------------------------
    v_tag = (te @ ipw[2 * H:].T + ipb[2 * H:]) @ opw.T + ob_   # [T, H]
    weff = np.stack(
        [f1w[:, t * H:(t + 1) * H] @ v_tag[t] for t in range(T)])
    weff_c = np.ascontiguousarray(
        (weff / H1_DIV).reshape(T, KC_H, P).astype(bfl))

    lwgT = (lw * lg).T                                   # [NEW_H, NL]
    lwg_np = np.zeros((P, KC_H, M_PR), bfl)              # we-part lhsT
    lwg_np[:, :, 0] = 1.0                                # sum row first
    lwg_np[:, :, 1:] = _chunked(lwgT[:H].astype(bfl), KC_H)
    c1n_np = np.zeros(M_PR, np.float16)
    c1n_np[1:] = (-lwgT.sum(0) / NEW_H).astype(np.float16)
    c2_np = np.zeros(M_PR, np.float16)
    c2_np[1:] = (lw @ lb + lbias).astype(np.float16)
    # fold lwg_h2.T @ ff2: the label projection reads relu(h1) directly
    lwg_h2 = np.concatenate(
        [np.ones((H2, 1), np.float32), lwgT[H:]], axis=1)    # [H2, 34]
    prh2_full = lwg_h2.T @ f2w                               # [34, H]
    cb_np = (lwg_h2.T @ f2b).astype(np.float16)              # [34]

    ff2t_np = np.zeros((P, KC_H, F8_W), f8)
    ff2t_np[:, :, :H2] = _chunked((f2w.T * FF2_SCALE).astype(f8), KC_H)
    ff2t_np[:, :, H2:] = _chunked(
        np.ascontiguousarray(prh2_full.T * H1_DIV).astype(f8), KC_H)
    ff1b_np = np.ascontiguousarray(f1b.reshape(KC_H, P).T) / H1_DIV
    ff2b_np = np.ascontiguousarray(f2b.reshape(KC_H2, P).T)

    counts_per_b = np.bincount(sb, minlength=B)
    nt = max(1, int(np.ceil(counts_per_b.max() / P)))
    n_pad = nt * P

    in_maps = []
    for c in range(NCORES):
        idx = np.where(sb == c)[0]
        n = len(idx)
        pk32 = np.zeros((P, 2 * nt + KC_H + KC_H2), np.float32)
        sps_np = np.zeros(n_pad, np.float32)
        spe_np = np.zeros(n_pad, np.float32)
        oht_np = np.zeros((n_pad, 2 * T), bfl)
        sps_np[:n] = ss[idx]
        spe_np[:n] = se[idx]
        oht_np[np.arange(n), st[idx]] = 1.0
        oht_np[np.arange(n), T + st[idx]] = -1.0
        pk32[:, 0:nt] = sps_np.reshape(nt, P).T
        pk32[:, nt:2 * nt] = spe_np.reshape(nt, P).T
        pk32[:, 2 * nt:2 * nt + KC_H] = ff1b_np
        pk32[:, 2 * nt + KC_H:] = ff2b_np
        oht_pk = np.ascontiguousarray(
            oht_np.reshape(nt, P, 2 * T).transpose(1, 0, 2)
            .reshape(P, nt * 2 * T))
        lwg_pk = np.ascontiguousarray(lwg_np.reshape(P, LWG_W))
        crow_pk = np.zeros((1, 3 * M_PR), np.float16)
        crow_pk[0, 0:M_PR] = c1n_np
        crow_pk[0, M_PR:2 * M_PR] = c2_np
        crow_pk[0, 2 * M_PR:] = cb_np
        in_maps.append(dict(
            pk32=pk32, oht=oht_pk, lwg=lwg_pk, crow=crow_pk,
            weff=weff_c, ff2t=ff2t_np,
            we_t=_chunked(np.ascontiguousarray(we[c].T).astype(bfl), KC_H),
        ))

    if nt not in _CACHE:
        _CACHE[nt] = build_kernel(nt)
    nc = _CACHE[nt]

    res = run_bass_kernel_spmd(nc, in_maps, list(range(NCORES)))
    out = np.stack([res.results[c]["out"].T for c in range(NCORES)])
    return out.astype(np.float32)


if __name__ == "__main__":
    import reference
    inp = {k: np.asarray(v) for k, v in reference.setup_inputs().items()}
    got = kernel(**inp)
    print("kernel output:", got.shape, got.dtype)


# revision 76
# speedup vs baseline: 1.1408x; 1.1226x over previous
"""Trainium2 Bass kernel for nn_Estor_concat (scatter_memory).

Math (exact reformulation of the reference):
  The attention output for a span of tag t is the per-tag constant
  v_tag[t] = out_proj(V_proj(tag_emb[t])) (softmax over one logit == 1),
  so the FFN input reduces to counts[t, s] * v_tag[t] concatenated over
  tags, and the first FFN layer collapses to the [T, H] weight-only
  constant W_eff[t, j] = sum_h v_tag[t, h] * ff1_w[j, t*H + h], folded on
  the host (constant folding, like BN-folding).  Per batch b the device
  computes:
    counts[t, s] = #spans(tag t) covering s
                 = sum_n oht[n,t]*(s >= start_n) - oht[n,t]*(s >= end_n)
    h1 = relu(W_eff.T @ counts + ff1_b)          [H, S]
    h2 = ff2 @ h1 + ff2_b                        [H2, S]  (fp8 DoubleRow)
    raw = [lwg_we | lwg_h2].T @ [we; h2]         [NL+1, S] (+ sum row)
    out = (raw - c1*mu + c2*sd) / bcast(sd)      (LayerNorm folded into
                                                  the output projection)
  with lwg = (lin_w * ln_g).T, c1 = col-sums of lwg, c2 = lin_w@ln_b+lin_b.

Sharding: pure data-parallel over batch (8 cores, 1 batch each), no
collectives; all post-fold weights are small and replicated.
"""

import ml_dtypes
import numpy as np

import concourse.bacc as bacc
import concourse.bass as bass
import concourse.mybir as mybir
import concourse.tile as tile
from concourse.bass_utils import run_bass_kernel_spmd

T, B, S, H = 16, 8, 512, 768
H2 = 384
NEW_H = H + H2          # 1152
NL = 33                 # num labels
NCORES = 8
KC_H = H // 128         # 6 chunks of the hidden dim
KC_H2 = H2 // 128       # 3
P = 128
M_PR = NL + 1           # 34: label rows + ones (sum) row
FF2_SCALE = 64.0        # fp8 pre-scale keeping ff2 out of e4m3 subnormals
H1_DIV = 4.0            # h1r stored /4 so prh2_w*4 clears fp8 subnormals
H2_SCALE = FF2_SCALE / H1_DIV   # h2 psum arrives scaled by this
LWG_W = KC_H * M_PR             # lwg_we chunks
F8_W = H2 + M_PR                # ff2 | prh2 packed width

F32 = mybir.dt.float32
BF16 = mybir.dt.bfloat16
F16 = mybir.dt.float16
F8 = mybir.dt.float8e4
DR = mybir.MatmulPerfMode.DoubleRow
ALU = mybir.AluOpType
ACT = mybir.ActivationFunctionType


def build_kernel(nt: int):
    nc = bacc.Bacc(
        "TRN2",
        target_bir_lowering=False,
        debug=False,
        enable_asserts=True,
        num_devices=NCORES,
    )

    def inp(name, shape, dtype=F32):
        return nc.dram_tensor(name, list(shape), dtype, kind="ExternalInput").ap()

    # packed inputs (few DMAs; see host prep for layouts)
    pk32 = inp("pk32", (P, 2 * nt + KC_H + KC_H2))  # sps | spe | ff1b | ff2b
    oht = inp("oht", (P, nt * 2 * T), BF16)         # [+onehot | -onehot]
    lwg = inp("lwg", (P, LWG_W), BF16)              # lwg_we lhsT chunks
    crow = inp("crow", (1, 3 * M_PR), F16)          # c1n | c2 | cb
    weff = inp("weff", (T, KC_H, P), BF16)          # W_eff[t, kj*128+m] / 4
    ff2t = inp("ff2t", (P, 3, 3 * 256 + 68), F8)    # DRSwInterleave packed
    we_t = inp("we_t", (P, KC_H, S), BF16)          # word_embedding[b].T

    out = nc.dram_tensor("out", [NL, S], F32, kind="ExternalOutput").ap()

    with tile.TileContext(nc) as tc:
        with (
            tc.tile_pool(name="singles", bufs=1) as singles,
            tc.tile_pool(name="spans", bufs=5) as spans,
            tc.tile_pool(name="ps_acc", bufs=1, space="PSUM") as ps_acc,
            tc.tile_pool(name="ps_h1", bufs=2, space="PSUM") as ps_h1,
            tc.tile_pool(name="ps_h2", bufs=2, space="PSUM") as ps_h2,
        ):
            # ---- constants ----
            ones_col = singles.tile([P, 1], BF16)
            nc.vector.memset(ones_col, 1.0)
            ones_row = singles.tile([1, M_PR], F16)
            nc.vector.memset(ones_row, 1.0)
            ones512 = singles.tile([1, S], F16)
            nc.vector.memset(ones512, 1.0)
            eps_t = singles.tile([1, 1], F32)
            nc.vector.memset(eps_t, 0.0)
            scratch = singles.tile([1, 1], F32)
            warm_sb = singles.tile([P, S], BF16)
            nc.gpsimd.memset(warm_sb, 0.25)
            # iota generated on-device: cheaper than a DMA (no 900ns sem)
            iota = singles.tile([P, S], F16)
            nc.gpsimd.iota(iota, [[1, S]], base=0, channel_multiplier=0,
                           allow_small_or_imprecise_dtypes=True)

            # ---- DMAs: mask-path + we lead the HWDGE queue; the small
            # weight tensors ride the Pool SWDGE queue in parallel ----
            pk32_sb = singles.tile([P, 2 * nt + KC_H + KC_H2], F32)
            nc.sync.dma_start(out=pk32_sb, in_=pk32)
            oht_sb = singles.tile([P, nt * 2 * T], BF16)
            nc.sync.dma_start(out=oht_sb, in_=oht)
            we_sb = singles.tile([P, KC_H, S], BF16)
            nc.sync.dma_start(out=we_sb[:, 0:3, :], in_=we_t[:, 0:3, :])
            nc.sync.dma_start(out=we_sb[:, 3:6, :], in_=we_t[:, 3:6, :])
            weff_sb = singles.tile([T, KC_H, P], BF16)
            nc.gpsimd.dma_start(out=weff_sb, in_=weff)
            lwg_sb = singles.tile([P, LWG_W], BF16)
            nc.gpsimd.dma_start(out=lwg_sb, in_=lwg)
            crow_sb = singles.tile([1, 3 * M_PR], F16)
            nc.gpsimd.dma_start(out=crow_sb, in_=crow)
            ff2_sb = singles.tile([P, 3, 3 * 256 + 68], F8)
            nc.gpsimd.dma_start(out=ff2_sb, in_=ff2t)

            def sps_col(i):
                return pk32_sb[:, i:i + 1]

            def spe_col(i):
                return pk32_sb[:, nt + i:nt + i + 1]

            def ff1b_col(kj):
                return pk32_sb[:, 2 * nt + kj:2 * nt + kj + 1]

            def ff2b_col(mc):
                return pk32_sb[:, 2 * nt + KC_H + mc:2 * nt + KC_H + mc + 1]

            def oht_pos(i):
                return oht_sb[:, i * 2 * T:i * 2 * T + T]

            def oht_neg(i):
                return oht_sb[:, i * 2 * T + T:(i + 1) * 2 * T]

            def lwg_c(fc):
                return lwg_sb[:, fc * M_PR:(fc + 1) * M_PR]

            c1n_row = crow_sb[0:1, 0:M_PR]
            c2_row = crow_sb[0:1, M_PR:2 * M_PR]
            cb_row = crow_sb[0:1, 2 * M_PR:3 * M_PR]

            # act-table warm-ups (loads overlap the DMA phase)
            nc.scalar.activation(out=scratch, in_=eps_t, func=ACT.Square)
            nc.scalar.activation(out=scratch, in_=eps_t, func=ACT.Sqrt,
                                 bias=eps_t)

            # ---- PE p-state warm-up: reach 2.4 GHz before real work ----
            warm_ps = ps_acc.tile([1, S], F32, tag="sdb")
            for _ in range(3):
                nc.tensor.matmul(warm_ps, ones_col, warm_sb,
                                 start=True, stop=True)

            # ---- counts: (s>=start) - (s>=end) scatter on PE ----
            counts_ps = ps_acc.tile([T, S], F32, tag="counts")
            for i in range(nt):
                ge_s = spans.tile([P, S], BF16, tag="ge_s")
                nc.vector.tensor_scalar(
                    out=ge_s, in0=iota, scalar1=sps_col(i), scalar2=None,
                    op0=ALU.is_ge,
                )
                ge_e = spans.tile([P, S], BF16, tag="ge_e")
                nc.vector.tensor_scalar(
                    out=ge_e, in0=iota, scalar1=spe_col(i), scalar2=None,
                    op0=ALU.is_ge,
                )
                nc.tensor.matmul(counts_ps, oht_pos(i), ge_s,
                                 start=(i == 0), stop=False)
                nc.tensor.matmul(counts_ps, oht_neg(i), ge_e,
                                 start=False, stop=(i == nt - 1))
            counts_sb = singles.tile([T, S], BF16)
            nc.vector.tensor_copy(out=counts_sb, in_=counts_ps)

            # ---- raw accumulation: constant cb opens the group ----
            pr_ps = ps_acc.tile([M_PR, S], F32, tag="pr")
            nc.tensor.matmul(pr_ps, cb_row, ones512, start=True, stop=False)
            for fc in range(3):
                nc.tensor.matmul(pr_ps, lwg_c(fc), we_sb[:, fc, :],
                                 start=False, stop=False)

            # ---- h1 = relu(W_eff.T @ counts + ff1_b) -> fp8 [H, S] ----
            h1r_sb = singles.tile([P, KC_H, S], F8)
            relu_eng = ["vec", "act", "vec", "act", "vec", "act"]
            h1_ps_l = []
            for kj in range(KC_H):
                if kj == 2:
                    # extra buffers via the freed counts and warm banks
                    ps = ps_acc.tile([P, S], F32, tag="counts")
                elif kj == 3:
                    ps = ps_acc.tile([P, S], F32, tag="sdb")
                else:
                    ps = ps_h1.tile([P, S], F32, tag="h1")
                h1_ps_l.append(ps)
                nc.tensor.matmul(ps, weff_sb[:, kj, :], counts_sb,
                                 start=True, stop=True)
            for kj in range(KC_H):
                ps = h1_ps_l[kj]
                if relu_eng[kj] == "vec":
                    nc.vector.tensor_scalar(
                        out=h1r_sb[:, kj, :], in0=ps, scalar1=ff1b_col(kj),
                        scalar2=0.0, op0=ALU.add, op1=ALU.max)
                elif relu_eng[kj] == "act":
                    nc.scalar.activation(
                        out=h1r_sb[:, kj, :], in_=ps, func=ACT.Relu,
                        bias=ff1b_col(kj))
                else:
                    nc.gpsimd.tensor_scalar(
                        out=h1r_sb[:, kj, :], in0=ps, scalar1=ff1b_col(kj),
                        scalar2=0.0, op0=ALU.add, op1=ALU.max)

            for fc in range(3, KC_H):
                nc.tensor.matmul(pr_ps, lwg_c(fc), we_sb[:, fc, :],
                                 start=False, stop=False)

            # ---- we squares ----
            sqp1 = singles.tile([P, 2, S], BF16)
            sqp2 = singles.tile([P, 2, S], BF16)
            sqw4 = singles.tile([P, S], BF16)
            sqw5 = singles.tile([P, S], BF16)
            nc.gpsimd.tensor_tensor(
                out=sqp1[:, 0, :], in0=we_sb[:, 0, :],
                in1=we_sb[:, 0, :], op=ALU.mult)
            nc.gpsimd.tensor_tensor(
                out=sqp1[:, 1, :], in0=we_sb[:, 1, :],
                in1=we_sb[:, 1, :], op=ALU.mult)
            nc.gpsimd.tensor_tensor(
                out=sqp2[:, 0, :], in0=we_sb[:, 2, :],
                in1=we_sb[:, 2, :], op=ALU.mult)
            nc.gpsimd.tensor_tensor(
                out=sqp2[:, 1, :], in0=we_sb[:, 3, :],
                in1=we_sb[:, 3, :], op=ALU.mult)
            nc.scalar.activation(
                out=sqw4, in_=we_sb[:, 4, :], func=ACT.Square)
            nc.scalar.activation(
                out=sqw5, in_=we_sb[:, 5, :], func=ACT.Square)

            # ---- sum of squares: tree-adds on the idle Pool engine ----
            a1 = singles.tile([P, S], BF16)
            nc.gpsimd.tensor_tensor(out=a1, in0=sqp1[:, 0, :],
                                    in1=sqp1[:, 1, :], op=ALU.add)
            a2 = singles.tile([P, S], BF16)
            nc.gpsimd.tensor_tensor(out=a2, in0=sqp2[:, 0, :],
                                    in1=sqp2[:, 1, :], op=ALU.add)
            a12 = singles.tile([P, S], BF16)
            nc.gpsimd.tensor_tensor(out=a12, in0=a1, in1=a2, op=ALU.add)
            a45 = singles.tile([P, S], BF16)
            nc.gpsimd.tensor_tensor(out=a45, in0=sqw4, in1=sqw5, op=ALU.add)
            acc_we = singles.tile([P, S], BF16)
            nc.gpsimd.tensor_tensor(out=acc_we, in0=a12, in1=a45,
                                    op=ALU.add)

            # ---- h2 = ff2 @ relu_h1 (fp8 DoubleRow) ----
            h2sqp = singles.tile([P, 3, S], BF16)
            h2_ps_l = []
            for mc in range(KC_H2):
                # third buffer for mc2 via the freed counts bank
                if mc == 2:
                    ps = ps_acc.tile([P, S], F32, tag="counts")
                else:
                    ps = ps_h2.tile([P, S], F32, tag="h2")
                h2_ps_l.append(ps)
            DRS = mybir.MatmulPerfMode.DoubleRowSwInterleave
            for mc in range(KC_H2):
                for pr_ in range(3):
                    nc.tensor.matmul(
                        h2_ps_l[mc],
                        ff2_sb[:, pr_, mc * 256:(mc + 1) * 256],
                        h1r_sb[:, 2 * pr_:2 * pr_ + 2, :],
                        start=(pr_ == 0), stop=(pr_ == 2),
                        perf_mode=DRS,
                    )
            # pr_h2 via folded fp8 weights: prh2_w.T @ relu(h1)
            for pr_ in range(3):
                nc.tensor.matmul(
                    pr_ps, ff2_sb[:, pr_, 768:768 + 2 * M_PR],
                    h1r_sb[:, 2 * pr_:2 * pr_ + 2, :],
                    start=False, stop=(pr_ == 2), perf_mode=DRS)

            # biased squares straight from psum (h2 values only feed the
            # variance; the label projection reads h1r via folded weights)
            nc.scalar.activation(
                out=h2sqp[:, 0, :], in_=h2_ps_l[0], func=ACT.Square,
                bias=ff2b_col(0), scale=1.0 / H2_SCALE)
            nc.scalar.activation(
                out=h2sqp[:, 1, :], in_=h2_ps_l[1], func=ACT.Square,
                bias=ff2b_col(1), scale=1.0 / H2_SCALE)
            nc.scalar.activation(
                out=h2sqp[:, 2, :], in_=h2_ps_l[2], func=ACT.Square,
                bias=ff2b_col(2), scale=1.0 / H2_SCALE)

            # ---- sum of squares, h2 part + single PE reduction ----
            b1 = singles.tile([P, S], BF16)
            nc.gpsimd.tensor_tensor(out=b1, in0=h2sqp[:, 0, :],
                                    in1=h2sqp[:, 1, :], op=ALU.add)
            b2 = singles.tile([P, S], BF16)
            nc.gpsimd.tensor_tensor(out=b2, in0=b1, in1=h2sqp[:, 2, :],
                                    op=ALU.add)
            comb = singles.tile([P, S], BF16)
            nc.vector.tensor_add(out=comb, in0=b2, in1=acc_we)
            ss_ps = ps_acc.tile([1, S], F32, tag="ss")
            nc.tensor.matmul(ss_ps, ones_col, comb, start=True, stop=True)

            # ---- LayerNorm stats: DVE back-to-back, no engine hops ----
            sumrow = singles.tile([1, S], F16)
            nc.vector.tensor_copy(out=sumrow, in_=pr_ps[0:1, :])
            # -c1*mu rides the pr psum accumulation (group re-opened)
            nc.tensor.matmul(pr_ps, c1n_row, sumrow,
                             start=False, stop=False, skip_group_check=True)
            mu = singles.tile([1, S], F16)
            nc.vector.tensor_scalar_mul(out=mu, in0=sumrow,
                                        scalar1=1.0 / NEW_H)
            mu2 = singles.tile([1, S], F16)
            nc.vector.tensor_mul(out=mu2, in0=mu, in1=mu)
            var_sb = singles.tile([1, S], F32)
            nc.vector.scalar_tensor_tensor(
                out=var_sb, in0=ss_ps, scalar=1.0 / NEW_H, in1=mu2,
                op0=ALU.mult, op1=ALU.subtract)
            sd = singles.tile([1, S], F16)
            nc.scalar.activation(out=sd, in_=var_sb, func=ACT.Sqrt,
                                 bias=eps_t)
            rvar = singles.tile([1, S], F32)
            nc.vector.reciprocal(out=rvar, in_=var_sb)
            # +c2*sd closes the pr group
            nc.tensor.matmul(pr_ps, c2_row, sd,
                             start=False, stop=True, skip_group_check=True)
            rstd = singles.tile([1, S], F32)
            nc.scalar.activation(out=rstd, in_=rvar, func=ACT.Sqrt,
                                 bias=eps_t)
            sdbs = singles.tile([M_PR, S], F32)
            nc.gpsimd.partition_broadcast(out_ap=sdbs, in_ap=rstd,
                                          channels=M_PR)

            # ---- final: (raw - c1*mu + c2*sd) / sd ----
            # asymmetric pieces on two DMA queues: the later piece is
            # smaller, so its divide (and the final semaphore) lands early
            HS = 416
            f_sa = singles.tile([M_PR, HS], F32)
            f_sb2 = singles.tile([M_PR, S - HS], F32)
            nc.vector.tensor_tensor(
                out=f_sa, in0=pr_ps[:, 0:HS],
                in1=sdbs[:, 0:HS], op=ALU.mult)
            nc.sync.dma_start(out=out[:, 0:HS], in_=f_sa[1:1 + NL, :])
            nc.vector.tensor_tensor(
                out=f_sb2, in0=pr_ps[:, HS:S],
                in1=sdbs[:, HS:S], op=ALU.mult)
            nc.scalar.dma_start(out=out[:, HS:S], in_=f_sb2[1:1 + NL, :])

    nc.compile()
    return nc


def _chunked(a, kc):
    """[kc*128, N...] -> [128, kc, N...] (partition-major chunk layout)."""
    return np.ascontiguousarray(
        a.reshape(kc, P, *a.shape[1:]).transpose(1, 0, *range(2, a.ndim + 1))
    )


_CACHE = {}


def kernel(**inputs) -> np.ndarray:
    bfl = ml_dtypes.bfloat16
    f8 = ml_dtypes.float8_e4m3
    we = np.asarray(inputs["word_embedding"], np.float32)
    te = np.asarray(inputs["tag_embedding"], np.float32)
    ipw = np.asarray(inputs["in_proj_w"], np.float32)
    ipb = np.asarray(inputs["in_proj_b"], np.float32)
    opw = np.asarray(inputs["out_proj_w"], np.float32)
    ob_ = np.asarray(inputs["out_proj_b"], np.float32)
    f1w = np.asarray(inputs["ff1_w"], np.float32)
    f1b = np.asarray(inputs["ff1_b"], np.float32)
    f2w = np.asarray(inputs["ff2_w"], np.float32)
    f2b = np.asarray(inputs["ff2_b"], np.float32)
    lg = np.asarray(inputs["ln_g"], np.float32)
    lb = np.asarray(inputs["ln_b"], np.float32)
    lw = np.asarray(inputs["lin_w"], np.float32)
    lbias = np.asarray(inputs["lin_b"], np.float32)
    sb = np.asarray(inputs["span_batch"]).astype(np.int64)
    st = np.asarray(inputs["span_tag"]).astype(np.int64)
    ss = np.asarray(inputs["span_start"]).astype(np.int64)
    se = np.asarray(inputs["span_end"]).astype(np.int64)

    # ---- weight-only constant folding (host) --------------------------
    v_tag = (te @ ipw[2 * H:].T + ipb[2 * H:]) @ opw.T + ob_   # [T, H]
    weff = np.stack(
        [f1w[:, t * H:(t + 1) * H] @ v_tag[t] for t in range(T)])
    weff_c = np.ascontiguousarray(
        (weff / H1_DIV).reshape(T, KC_H, P).astype(bfl))

    lwgT = (lw * lg).T                                   # [NEW_H, NL]
    lwg_np = np.zeros((P, KC_H, M_PR), bfl)              # we-part lhsT
    lwg_np[:, :, 0] = 1.0                                # sum row first
    lwg_np[:, :, 1:] = _chunked(lwgT[:H].astype(bfl), KC_H)
    c1n_np = np.zeros(M_PR, np.float16)
    c1n_np[1:] = (-lwgT.sum(0) / NEW_H).astype(np.float16)
    c2_np = np.zeros(M_PR, np.float16)
    c2_np[1:] = (lw @ lb + lbias).astype(np.float16)
    # fold lwg_h2.T @ ff2: the label projection reads relu(h1) directly
    lwg_h2 = np.concatenate(
        [np.ones((H2, 1), np.float32), lwgT[H:]], axis=1)    # [H2, 34]
    prh2_full = lwg_h2.T @ f2w                               # [34, H]
    cb_np = (lwg_h2.T @ f2b).astype(np.float16)              # [34]

    ff2_ch = np.zeros((P, KC_H, F8_W), np.float32)
    ff2_ch[:, :, :H2] = _chunked(f2w.T * FF2_SCALE, KC_H)
    ff2_ch[:, :, H2:] = _chunked(
        np.ascontiguousarray(prh2_full.T * H1_DIV), KC_H)

    def _ilv(a, b):
        # [P, M] x2 -> [P, 2M]: A/B interleaved per column, columns reversed
        return np.stack([a[:, ::-1], b[:, ::-1]], axis=2).reshape(P, -1)

    ff2t_np = np.zeros((P, 3, 3 * 256 + 68), f8)
    for pr_ in range(3):
        for mc in range(KC_H2):
            ff2t_np[:, pr_, mc * 256:(mc + 1) * 256] = _ilv(
                ff2_ch[:, 2 * pr_, mc * P:(mc + 1) * P],
                ff2_ch[:, 2 * pr_ + 1, mc * P:(mc + 1) * P]).astype(f8)
        ff2t_np[:, pr_, 768:] = _ilv(
            ff2_ch[:, 2 * pr_, H2:], ff2_ch[:, 2 * pr_ + 1, H2:]).astype(f8)
    ff1b_np = np.ascontiguousarray(f1b.reshape(KC_H, P).T) / H1_DIV
    ff2b_np = np.ascontiguousarray(f2b.reshape(KC_H2, P).T)

    counts_per_b = np.bincount(sb, minlength=B)
    nt = max(1, int(np.ceil(counts_per_b.max() / P)))
    n_pad = nt * P

    in_maps = []
    for c in range(NCORES):
        idx = np.where(sb == c)[0]
        n = len(idx)
        pk32 = np.zeros((P, 2 * nt + KC_H + KC_H2), np.float32)
        sps_np = np.zeros(n_pad, np.float32)
        spe_np = np.zeros(n_pad, np.float32)
        oht_np = np.zeros((n_pad, 2 * T), bfl)
        sps_np[:n] = ss[idx]
        spe_np[:n] = se[idx]
        oht_np[np.arange(n), st[idx]] = 1.0
        oht_np[np.arange(n), T + st[idx]] = -1.0
        pk32[:, 0:nt] = sps_np.reshape(nt, P).T
        pk32[:, nt:2 * nt] = spe_np.reshape(nt, P).T
        pk32[:, 2 * nt:2 * nt + KC_H] = ff1b_np
        pk32[:, 2 * nt + KC_H:] = ff2b_np
        oht_pk = np.ascontiguousarray(
            oht_np.reshape(nt, P, 2 * T).transpose(1, 0, 2)
            .reshape(P, nt * 2 * T))
        lwg_pk = np.ascontiguousarray(lwg_np.reshape(P, LWG_W))
        crow_pk = np.zeros((1, 3 * M_PR), np.float16)
        crow_pk[0, 0:M_PR] = c1n_np
        crow_pk[0, M_PR:2 * M_PR] = c2_np
        crow_pk[0, 2 * M_PR:] = cb_np
        in_maps.append(dict(
            pk32=pk32, oht=oht_pk, lwg=lwg_pk, crow=crow_pk,
            weff=weff_c, ff2t=ff2t_np,
            we_t=_chunked(np.ascontiguousarray(we[c].T).astype(bfl), KC_H),
        ))

    if nt not in _CACHE:
        _CACHE[nt] = build_kernel(nt)
    nc = _CACHE[nt]

    res = run_bass_kernel_spmd(nc, in_maps, list(range(NCORES)))
    out = np.stack([res.results[c]["out"].T for c in range(NCORES)])
    return out.astype(np.float32)


if __name__ == "__main__":
    import reference
    inp = {k: np.asarray(v) for k, v in reference.setup_inputs().items()}
    got = kernel(**inp)
    print("kernel output:", got.shape, got.dtype)
